# revision 21
# baseline (speedup 1.0000x reference)
"""Biaffine edge attention on 8 Trainium2 NeuronCores.

Math (per batch b):
    out[i,o] = head[i,:] @ U @ dep[o,:] + head[i,:]@wh + dep[o,:]@wd + b
with head/dep [S=2048, D=256], U [D,D], edge_W = [wh | wd] (each [D]).

Sharding: pure data-parallel over batch B=8 -> one batch per core,
U / edge_W / edge_b replicated. No collectives.

Per-core kernel:
    ATf[e,i] = sum_d U[d,e] * headT[d,i] + wd[e]      (dep-side rank-1 term
               rides the e-contraction for free)
    hs[i]    = sum_d head[i,d] * wh[d]  + b           (GpSimd mul + DVE reduce)
    out[i,o] = sum_e ATf[e,i] * depT[e,o]  + hs[i]

exec_time ~= first_store_time + 16.8MB / ~420GB/s + wrapup, so the whole
schedule optimizes when the first out bytes hit the store ring:
  * head is loaded in "16-consecutive-rows-per-partition" layout
    (partition p holds rows 16p..16p+15) -> 4-8KB contiguous DMA
    descriptors.  Out tile r then covers rows {16p + r}, an affine DRAM
    AP with 8KB-contiguous rows, and hs lines up as hs_colb[p, r].
    dep keeps the 128-row-block layout (its transpose must produce
    natural o-order for contiguous stores).
  * Load order (all triggers upfront, ACT HWDGE ring): eye, head-half0
    in 2 quads, U/wh consts, dep half0, dep half1, head half1 -- sized
    so the first-store dependency chain (transpose -> collect -> AT ->
    mm -> epilogue) starts as early as possible.
  * Stores go on the sync (SP) HWDGE ring, independent of the loads.
    Tiles r0..7 store as [128, 2 rows x 1024 cols] mega-chunks (4KB
    descriptors) chasing the dep halves; r8..15 as full rows.
  * Emission order == Tile-scheduler priority: the exact critical chain
    to the first store is emitted first; distractor work (dep half1,
    head half1, late hs) carries tile_wait_until floors so the greedy
    scheduler cannot slot it into the critical window.
Matmuls run as float32r (1 cycle/row for moving dim >= 256).  FP32r
matmul inputs must be produced by a compute op, so matmul-feeding SBUF
tiles are float32r-typed and written by DVE/ACT copies, never by DMA.
"""

import numpy as np

import concourse.bass as bass
import concourse.tile as tile
from concourse import bacc, mybir
from concourse.bass_utils import run_bass_kernel_spmd

B, S, D = 8, 2048, 256
P = 128          # partitions
OC = 512         # matmul output free-dim chunk (one PSUM bank of fp32)
RPP = 16         # head rows per partition (out tile r covers rows 16p+r)
NI = S // P      # 16 out row-tiles
NO = S // OC     # 4 output column chunks
ND = D // P      # 2 contraction chunks
HH = 2           # input halves
RPH = 8          # r-blocks (head) / j-blocks (dep) per half
F32 = mybir.dt.float32
F32R = mybir.dt.float32r

# const pack 1 layout: U0 | U1 | wh | wdT | b
C_U0, C_U1, C_WH, C_WDT, C_B = 0, D, 2 * D, 3 * D, 3 * D + ND
C_TOT = C_B + 1


def build_nc(reps=1):
    nc = bacc.Bacc("TRN2", target_bir_lowering=False, debug=False, num_devices=B)

    head_d = nc.dram_tensor("head", [S, D], F32, kind="ExternalInput")
    dep_d = nc.dram_tensor("dep", [S, D], F32, kind="ExternalInput")
    eye_d = nc.dram_tensor("eye", [P, P], F32, kind="ExternalInput")
    cst_d = nc.dram_tensor("cpack", [P, C_TOT], F32, kind="ExternalInput")
    out_d = nc.dram_tensor("out", [S, S], F32, kind="ExternalOutput")

    Ident = mybir.ActivationFunctionType.Identity

    with tile.TileContext(nc) as tc:
        with (
            tc.tile_pool(name="const", bufs=1) as cpool,
            tc.tile_pool(name="persist", bufs=1) as ppool,
            tc.tile_pool(name="ttrp", bufs=2) as ttrp,
            tc.tile_pool(name="pairbuf", bufs=4) as pairbuf,
            tc.tile_pool(name="outbuf", bufs=4) as outbuf,
            tc.tile_pool(name="ps_t", bufs=3, space=bass.MemorySpace.PSUM) as ps_t,
            tc.tile_pool(name="ps_mm", bufs=5, space=bass.MemorySpace.PSUM) as ps_mm,
        ):
            # ---- all loads issued upfront on the ACT HWDGE ring, in the
            # order the compute chain consumes them ----
            eye_t = cpool.tile([P, P], F32, name="eye", tag="eye")
            nc.scalar.dma_start(eye_t[:], eye_d[:])
            eye = eye_t[:]

            head3 = head_d[0:S, :].rearrange("(p rr) d -> p rr d", rr=RPP)
            nat_h = [ppool.tile([P, RPH * D], F32, name=f"nath{hh}", tag=f"nath{hh}")
                     for hh in range(HH)]
            # head half0 split into 2 quad loads so transposes start sooner
            for q in range(2):
                nc.scalar.dma_start(
                    nat_h[0][:, q * 4 * D:(q + 1) * 4 * D].rearrange(
                        "p (r d) -> p r d", d=D),
                    head3[:, q * 4:(q + 1) * 4, :],
                )

            cst = cpool.tile([P, C_TOT], F32, name="cst", tag="cst")
            nc.scalar.dma_start(cst[:], cst_d[:])

            # dep halves: 8 row-blocks each, aligned with store column-halves
            nat_dp = [ppool.tile([P, RPH * D], F32, name=f"natd{hh}", tag=f"natd{hh}")
                      for hh in range(HH)]
            for hh in range(HH):
                src = dep_d[hh * RPH * P:(hh + 1) * RPH * P, :]
                nc.scalar.dma_start(
                    nat_dp[hh][:].rearrange("p (j d) -> p j d", d=D),
                    src.rearrange("(j p) d -> p j d", p=P),
                )

            nc.scalar.dma_start(
                nat_h[1][:].rearrange("p (r d) -> p r d", d=D),
                head3[:, RPH:RPP, :],
            )

            # ---- f32r copies of U (DVE, right after cst lands) ----
            u_sb = []
            for dc in range(ND):
                u_t = cpool.tile([P, D], F32R, name=f"u{dc}", tag=f"u{dc}")
                nc.vector.tensor_copy(u_t[:], cst[:, C_U0 + dc * D:C_U0 + (dc + 1) * D])
                u_sb.append(u_t)

            # ---- persistent SBUF tensors ----
            headT = [ppool.tile([P, S], F32R, name=f"headT{dc}", tag=f"headT{dc}")
                     for dc in range(ND)]
            depT = [ppool.tile([P, S], F32R, name=f"depT{dc}", tag=f"depT{dc}")
                    for dc in range(ND)]
            atf = [ppool.tile([P, S], F32R, name=f"atf{eb}", tag=f"atf{eb}")
                   for eb in range(ND)]
            hs_colb = ppool.tile([P, NI], F32, name="hs_colb", tag="hs_colb")

            out3 = out_d[0:S, :].rearrange("(p rr) o -> p rr o", rr=RPP)

            def transpose_quad(nat, dstT, qg):
                # 8 PE transposes of one 4-block quad -> 2 [128,512] collect
                # copies (DVE dc0 / ACT dc1).  qg = global quad index 0..3.
                for dc in range(ND):
                    pst = ps_t.tile([P, 4 * P], F32, name="pst", tag="pst")
                    for k in range(4):
                        blk = (qg % 2) * 4 + k
                        nc.tensor.transpose(
                            pst[:, k * P:(k + 1) * P],
                            nat[:, blk * D + dc * P: blk * D + dc * P + P],
                            eye,
                        )
                    dst = dstT[dc][:, qg * 4 * P:(qg + 1) * 4 * P]
                    if dc == 0:
                        nc.vector.tensor_copy(dst, pst[:])
                    else:
                        nc.scalar.copy(dst, pst[:])

            def at_quad(qg):
                # ATf for one 512-i-column quad; eb0 copy on ACT, eb1 on DVE
                c0 = qg * OC
                for eb in range(ND):
                    pa = ps_mm.tile([P, OC], F32, name="psmm", tag="psmm")
                    for dc in range(ND):
                        nc.tensor.matmul(
                            pa[:],
                            u_sb[dc][:, eb * P:(eb + 1) * P],
                            headT[dc][:, c0:c0 + OC],
                            start=(dc == 0),
                            stop=(dc == ND - 1),
                        )
                    wdb = cst[:, C_WDT + eb:C_WDT + eb + 1]
                    if eb == 0:
                        nc.scalar.activation(
                            atf[eb][:, c0:c0 + OC], pa[:], Ident, bias=wdb)
                    else:
                        nc.vector.tensor_scalar_add(
                            atf[eb][:, c0:c0 + OC], pa[:], wdb)

            def hs_block(r):
                # hs_colb[p, r] = b + sum_d nat_h[p, r*D+d] * wh[d] -- mul on
                # the otherwise-idle GpSimd, cheap X-reduce (+bias) on DVE.
                hh, rloc = r // RPH, r % RPH
                ttr = ttrp.tile([P, D], F32, name="ttr", tag="ttr")
                nc.gpsimd.tensor_mul(
                    ttr[:], nat_h[hh][:, rloc * D:(rloc + 1) * D],
                    cst[:, C_WH:C_WH + D],
                )
                hsr = ttrp.tile([P, 1], F32, name="hsr", tag="hsr")
                nc.vector.reduce_sum(hsr[:], ttr[:], axis=mybir.AxisListType.X)
                nc.vector.tensor_scalar_add(
                    hs_colb[:, r:r + 1], hsr[:], cst[:, C_B:C_B + 1],
                )

            def mm_chunk(r, oc, dst, eng):
                po = ps_mm.tile([P, OC], F32, name="psmm", tag="psmm")
                for eb in range(ND):
                    nc.tensor.matmul(
                        po[:],
                        atf[eb][:, r * P:(r + 1) * P],
                        depT[eb][:, oc * OC:(oc + 1) * OC],
                        start=(eb == 0),
                        stop=(eb == ND - 1),
                    )
                if eng == 0:
                    nc.scalar.activation(dst, po[:], Ident, bias=hs_colb[:, r:r + 1])
                else:
                    nc.vector.tensor_scalar_add(dst, po[:], hs_colb[:, r:r + 1])

            def pair_store(pr, och, engs):
                # [128, 2 rows x 1024 cols] mega-chunk store (4KB descriptors)
                pt = pairbuf.tile([P, 2 * 2 * OC], F32, name="pt", tag="pt")
                for half in range(2):
                    r = pr * 2 + half
                    for ocw in range(2):
                        mm_chunk(r, och * 2 + ocw, eng=engs[half * 2 + ocw],
                                 dst=pt[:, half * 2 * OC + ocw * OC:
                                        half * 2 * OC + (ocw + 1) * OC])
                nc.sync.dma_start(
                    out3[:, pr * 2:pr * 2 + 2,
                         och * 2 * OC:(och + 1) * 2 * OC],
                    pt[:].rearrange("p (rr o) -> p rr o", rr=2),
                )

            def body():
                # critical chain to the first store, in priority order
                transpose_quad(nat_h[0], headT, 0)
                at_quad(0)
                for r in range(4):
                    hs_block(r)
                transpose_quad(nat_h[0], headT, 1)
                at_quad(1)
                for r in range(4, RPH):
                    hs_block(r)
                for q in range(2):
                    transpose_quad(nat_dp[0], depT, q)
                # phase A half 0: tiles r0..7 x cols 0..1023
                pair_store(0, 0, engs=(0, 1, 0, 1))   # first store: 2 ACT+2 DVE
                pair_store(1, 0, engs=(1,) * 4)
                pair_store(2, 0, engs=(0,) * 4)
                pair_store(3, 0, engs=(1,) * 4)
                # dep half1 + phase A half 1 (floored out of the early window)
                with tc.tile_wait_until(0.012):
                    for q in range(2):
                        transpose_quad(nat_dp[1], depT, 2 + q)
                for pr in range(4):
                    pair_store(pr, 1, engs=((pr + 1) % 2,) * 4)
                # head half1 chains + phase B
                with tc.tile_wait_until(0.016):
                    transpose_quad(nat_h[1], headT, 2)
                    at_quad(2)
                    transpose_quad(nat_h[1], headT, 3)
                    at_quad(3)
                    for r in range(RPH, NI):
                        hs_block(r)
                for r in range(RPH, NI):
                    ot = outbuf.tile([P, S], F32, name="ot", tag="ot")
                    for oc in range(NO):
                        mm_chunk(r, oc, ot[:, oc * OC:(oc + 1) * OC], eng=r % 2)
                    nc.sync.dma_start(out3[:, r:r + 1, :],
                                      ot[:].rearrange("p (rr o) -> p rr o", rr=1))

            if reps > 1:
                with tc.For_i(0, reps, 1):
                    body()
            else:
                body()

    nc.finalize()
    return nc


_NC_CACHE = {}


def _get_nc(reps=1):
    if reps not in _NC_CACHE:
        _NC_CACHE[reps] = build_nc(reps)
    return _NC_CACHE[reps]


def make_in_maps(head, dep, edge_U, edge_W, edge_b):
    head = np.ascontiguousarray(np.asarray(head, dtype=np.float32))
    dep = np.ascontiguousarray(np.asarray(dep, dtype=np.float32))
    u = np.asarray(edge_U, dtype=np.float32)
    w = np.asarray(edge_W, dtype=np.float32).reshape(-1)
    wh, wd = w[:D], w[D:]
    bval = float(np.asarray(edge_b).reshape(-1)[0])

    eye = np.eye(P, dtype=np.float32)
    cpack = np.zeros((P, C_TOT), dtype=np.float32)
    cpack[:, C_U0:C_U0 + D] = u[0:P, :]
    cpack[:, C_U1:C_U1 + D] = u[P:2 * P, :]
    cpack[:, C_WH:C_WH + D] = np.tile(wh[None, :], (P, 1))
    cpack[:, C_WDT:C_WDT + ND] = wd.reshape(ND, P).T
    cpack[:, C_B] = bval
    cpack = np.ascontiguousarray(cpack)

    return [
        {"head": head[b], "dep": dep[b], "eye": eye, "cpack": cpack}
        for b in range(B)
    ]


def kernel(head, dep, edge_U, edge_W, edge_b):
    nc = _get_nc()
    in_maps = make_in_maps(head, dep, edge_U, edge_W, edge_b)
    res = run_bass_kernel_spmd(nc, in_maps, core_ids=list(range(B)))
    return np.stack([res.results[b]["out"] for b in range(B)], axis=0)
